# revision 14
# baseline (speedup 1.0000x reference)
"""Multi-head self-attention (post-softmax gauss reweight variant) on 8 TRN2 cores.

Sharding: core c handles batch b = c//2 and query-row half r = c%2 (512 rows),
all 16 heads. No cross-core communication.

Math (per batch b):
  q = (query @ Wq + bq) / 8 ;  k = key @ Wk + bk ;  v = value @ Wv + bv
  softmax -> gauss reweight -> renormalize collapses to a single
  normalization:  w_qk = exp(s_qk + lnG_k) / sum_k exp(s_qk + lnG_k)
  where lnG_k = ln(gauss_k + 1e-10) + (mask_k==0 ? -1e9 : 0); the softmax
  denominator cancels against the renormalization.
  out = (w @ v) @ Wo + bo

Device layout: everything is computed in "transposed" orientation
(dims on partitions, sequence on free axis):
  qT = Wq^T @ query^T  (per 128-dim tile, PE matmul, fp32r)
  kT likewise;  v in normal [kpos, dh] orientation (lhsT = value^T).
  sT[kpos, q] = kT' . qT' per head (K=64, two heads row-packed in the PE).
  p = exp(sT + lnG) on ACT (bias is per-partition in this orientation).
  PV: lhsT = [v_h | 1] (M=65) -> psC[0:64]=ctx^T, psC[64]=denominator.
  Normalize via rank-1 ones x recip(den) matmul + DVE multiply.
  out = ctx^T.T @ Wo + bo (rank-1 ones x bias for the free-axis biases).
"""

import os
import sys
import types

sys.path.insert(0, "/opt/trn_rl_repo")

import numpy as np

# The agent image's antenv package lacks axon_hooks, so trn_boot's NTFF hook
# registration silently degrades. Recreate the module so
# run_bass_kernel_spmd(trace=True) can profile (used by test.py; harmless
# otherwise).
try:
    import antenv

    if "antenv.axon_hooks" not in sys.modules:
        _hooks_mod = types.ModuleType("antenv.axon_hooks")
        _hooks_mod._hook = None
        _hooks_mod.set_axon_ntff_profile_hook = lambda h: setattr(
            _hooks_mod, "_hook", h
        )
        _hooks_mod.get_axon_ntff_profile_hook = lambda: _hooks_mod._hook
        sys.modules["antenv.axon_hooks"] = _hooks_mod
        antenv.axon_hooks = _hooks_mod
        try:
            from trn_agent_boot.trn_boot import _ntff_profile_via_ctypes

            _hook = _ntff_profile_via_ctypes("/opt/axon/libaxon_pjrt.so")
            if _hook is not None:
                _hooks_mod.set_axon_ntff_profile_hook(_hook)
        except Exception:
            pass
except Exception:
    pass

import concourse.bass as bass
import concourse.mybir as mybir
import concourse.tile as tile
from concourse import bacc
from concourse import bass_utils

BS, SEQ, DIM, H = 4, 1024, 1024, 16
DH = DIM // H  # 64
QH = SEQ // 2  # 512 rows of q per core
N_CORES = 8
KT = DIM // 128  # 8 contraction tiles
PT = SEQ // 128  # 8 kpos tiles
NPAIR = H // 2  # 8 head pairs

F32 = mybir.dt.float32
F32R = mybir.dt.float32r
I32 = mybir.dt.int32
AF = mybir.ActivationFunctionType

_CACHED = {}
LAST_RESULT = None


def _build():
    nc = bacc.Bacc("TRN2", target_bir_lowering=False, debug=False, num_devices=N_CORES)

    qT = nc.dram_tensor("qT", [DIM, QH], F32R, kind="ExternalInput").ap()
    kT = nc.dram_tensor("kT", [DIM, SEQ], F32R, kind="ExternalInput").ap()
    vT = nc.dram_tensor("vT", [DIM, SEQ], F32R, kind="ExternalInput").ap()
    Wq = nc.dram_tensor("Wq", [DIM, DIM], F32R, kind="ExternalInput").ap()
    Wk = nc.dram_tensor("Wk", [DIM, DIM], F32R, kind="ExternalInput").ap()
    Wv = nc.dram_tensor("Wv", [DIM, DIM], F32R, kind="ExternalInput").ap()
    Wo = nc.dram_tensor("Wo", [DIM, DIM], F32R, kind="ExternalInput").ap()
    bq = nc.dram_tensor("bq", [DIM], F32, kind="ExternalInput").ap()
    bk = nc.dram_tensor("bk", [DIM], F32, kind="ExternalInput").ap()
    bv = nc.dram_tensor("bv", [DIM], F32R, kind="ExternalInput").ap()
    bo = nc.dram_tensor("bo", [DIM], F32R, kind="ExternalInput").ap()
    gauss = nc.dram_tensor("gauss", [SEQ], F32, kind="ExternalInput").ap()
    mask = nc.dram_tensor("mask", [SEQ], I32, kind="ExternalInput").ap()
    out = nc.dram_tensor("out", [QH, DIM], F32, kind="ExternalOutput").ap()

    with tile.TileContext(nc) as tc:
        with (
            tc.tile_pool(name="const", bufs=1) as constp,
            tc.tile_pool(name="small", bufs=1) as smallp,
            tc.tile_pool(name="w", bufs=16) as wp,
            tc.tile_pool(name="xin", bufs=12) as xinp,
            tc.tile_pool(name="qtp", bufs=8) as qtpp,
            tc.tile_pool(name="ktp", bufs=8) as ktpp,
            tc.tile_pool(name="vsb", bufs=8) as vsbp,
            tc.tile_pool(name="psb", bufs=6) as psbp,
            tc.tile_pool(name="ctx", bufs=8) as ctxp,
            tc.tile_pool(name="norm", bufs=2) as normp,
            tc.tile_pool(name="osb", bufs=2) as osbp,
            tc.tile_pool(name="acc", bufs=2, space="PSUM") as accp,
            tc.tile_pool(name="sps", bufs=3, space="PSUM") as spsp,
            tc.tile_pool(name="cps", bufs=2, space="PSUM") as cpsp,
            tc.tile_pool(name="dps", bufs=1, space="PSUM") as dpsp,
        ):
            # ---- constants / small tensors ----
            ones_f = constp.tile([128, 128], F32)
            nc.gpsimd.memset(ones_f[:], 1.0)
            ones = constp.tile([128, 128], F32R)
            nc.vector.tensor_copy(ones[:], ones_f[:])
            # head-pair selector rows: selA = [1]*64+[0]*64, selB = [0]*64+[1]*64.
            # psD = selA^T@recA + selB^T@recB broadcasts each head's 1/den to
            # its 64-partition half via two full-width K=1 matmuls
            # (partial-partition-out matmuls fail the walrus ISA check).
            selA_f = constp.tile([1, 128], F32)
            nc.gpsimd.memset(selA_f[:], 0.0)
            nc.gpsimd.memset(selA_f[0:1, 0:64], 1.0)
            selB_f = constp.tile([1, 128], F32)
            nc.gpsimd.memset(selB_f[:], 0.0)
            nc.gpsimd.memset(selB_f[0:1, 64:128], 1.0)
            selA = constp.tile([1, 128], F32R)
            nc.vector.tensor_copy(selA[:], selA_f[:])
            selB = constp.tile([1, 128], F32R)
            nc.vector.tensor_copy(selB[:], selB_f[:])

            g_sb = smallp.tile([128, PT], F32)
            nc.sync.dma_start(out=g_sb[:], in_=gauss.rearrange("(t p) -> p t", p=128))
            m_i = smallp.tile([128, PT], I32)
            nc.sync.dma_start(out=m_i[:], in_=mask.rearrange("(t p) -> p t", p=128))
            m_f = smallp.tile([128, PT], F32)
            nc.vector.tensor_copy(m_f[:], m_i[:])
            # lnG = ln(gauss + 1e-10) + (mask - 1) * 1e9
            eps_t = smallp.tile([128, 1], F32)
            nc.gpsimd.memset(eps_t[:], 1e-10)
            lnG = smallp.tile([128, PT], F32)
            nc.scalar.activation(lnG[:], g_sb[:], AF.Ln, bias=eps_t[:, 0:1], scale=1.0)
            pen = smallp.tile([128, PT], F32)
            nc.vector.tensor_scalar(
                pen[:], m_f[:], 1e9, -1e9, mybir.AluOpType.mult, mybir.AluOpType.add
            )
            nc.vector.tensor_add(lnG[:], lnG[:], pen[:])

            bqs = smallp.tile([128, KT], F32)
            nc.sync.dma_start(out=bqs[:], in_=bq.rearrange("(t p) -> p t", p=128))
            nc.vector.tensor_scalar_mul(bqs[:], bqs[:], 0.125)
            bks = smallp.tile([128, KT], F32)
            nc.sync.dma_start(out=bks[:], in_=bk.rearrange("(t p) -> p t", p=128))
            bv_sb = smallp.tile([1, DIM], F32R)
            nc.sync.dma_start(out=bv_sb[:], in_=bv.rearrange("(a d) -> a d", a=1))
            bo_sb = smallp.tile([1, DIM], F32R)
            nc.sync.dma_start(out=bo_sb[:], in_=bo.rearrange("(a d) -> a d", a=1))

            # Weight/input streaming: all staged as [128, 512] tiles in rolling
            # pools, loop orders chosen so at most 8 tiles of a tag-phase are
            # live at once (kT/vT halves are loaded twice to keep it so).
            def load_w(src, t, h):
                wt = wp.tile([128, 512], F32R, tag="w")
                nc.sync.dma_start(
                    out=wt[:], in_=src[128 * t : 128 * (t + 1), 512 * h : 512 * (h + 1)]
                )
                return wt

            def load_x(src, t, h):
                ti = xinp.tile([128, 512], F32R, tag="xin")
                nc.sync.dma_start(
                    out=ti[:], in_=src[128 * t : 128 * (t + 1), 512 * h : 512 * (h + 1)]
                )
                return ti

            # ---- q projection: qTp[j] = (Wq^T @ queryT)[128j:, :] * 0.125 + bq*0.125
            qt_in = [load_x(qT, t, 0) for t in range(KT)]
            qTp = []
            for h in range(2):
                wq_sb = [load_w(Wq, t, h) for t in range(KT)]
                for j in range(4 * h, 4 * h + 4):
                    ps = accp.tile([128, QH], F32)
                    for t in range(KT):
                        nc.tensor.matmul(
                            ps[:],
                            wq_sb[t][:, 128 * (j % 4) : 128 * (j % 4 + 1)],
                            qt_in[t][:],
                            start=(t == 0),
                            stop=(t == KT - 1),
                        )
                    tj = qtpp.tile([128, QH], F32R, tag="qtp")
                    nc.scalar.activation(
                        tj[:], ps[:], AF.Identity, bias=bqs[:, j : j + 1], scale=0.125
                    )
                    qTp.append(tj)

            # ---- k projection: kTp[j][:, n] = (Wk^T @ keyT)[128j:, :] + bk
            kTp = [
                ktpp.tile([128, SEQ], F32R, tag="ktp", name=f"kTp{j}")
                for j in range(KT)
            ]
            for h in range(2):
                wk_sb = [load_w(Wk, t, h) for t in range(KT)]
                for n in range(2):
                    kt_in = [load_x(kT, t, n) for t in range(KT)]
                    for j in range(4 * h, 4 * h + 4):
                        ps = accp.tile([128, 512], F32)
                        for t in range(KT):
                            nc.tensor.matmul(
                                ps[:],
                                wk_sb[t][:, 128 * (j % 4) : 128 * (j % 4 + 1)],
                                kt_in[t][:],
                                start=(t == 0),
                                stop=(t == KT - 1),
                            )
                        nc.scalar.activation(
                            kTp[j][:, 512 * n : 512 * (n + 1)],
                            ps[:],
                            AF.Identity,
                            bias=bks[:, j : j + 1],
                            scale=1.0,
                        )

            # ---- v projection (normal orientation): v[m] = [kpos-tile, 16*(64+1)]
            # column 64 of each 65-wide head block is the all-ones denominator col.
            v_sb = []
            for m in range(PT):
                vm = vsbp.tile([128, H * (DH + 1)], F32R, tag="vsb")
                vv = vm[:].rearrange("p (h c) -> p h c", c=DH + 1)
                nc.vector.tensor_copy(vv[:, :, DH : DH + 1], ones_f[:, 0:H])
                v_sb.append(vm)
            for n in range(2):
                wv_sb = [load_w(Wv, t, n) for t in range(KT)]
                for mh in range(2):
                    vt_in = [load_x(vT, t, mh) for t in range(KT)]
                    for m in range(4 * mh, 4 * mh + 4):
                        ps = accp.tile([128, 512], F32)
                        for t in range(KT):
                            nc.tensor.matmul(
                                ps[:],
                                vt_in[t][:, 128 * (m % 4) : 128 * (m % 4 + 1)],
                                wv_sb[t][:],
                                start=(t == 0),
                                stop=False,
                            )
                        nc.tensor.matmul(
                            ps[:],
                            ones[0:1, 0:128],
                            bv_sb[0:1, 512 * n : 512 * (n + 1)],
                            start=False,
                            stop=True,
                        )
                        vv = v_sb[m][:].rearrange("p (h c) -> p h c", c=DH + 1)
                        nc.vector.tensor_copy(
                            vv[:, 8 * n : 8 * (n + 1), 0:DH],
                            ps[:].rearrange("p (h c) -> p h c", c=DH),
                        )

            # ---- attention, one head pair (2p, 2p+1) at a time ----
            ctx_sb = []
            for p in range(NPAIR):
                cx = ctxp.tile([128, QH], F32R, tag="ctx")
                ctx_sb.append(cx)
            for p in range(NPAIR):
                psC_A = cpsp.tile([65, QH], F32, tag="cps")
                psC_B = cpsp.tile([65, QH], F32, tag="cps")
                for t in range(PT):
                    psA = spsp.tile([128, QH], F32, tag="sps")
                    psB = spsp.tile([128, QH], F32, tag="sps")
                    nc.tensor.matmul(
                        psA[:],
                        kTp[p][0:64, 128 * t : 128 * (t + 1)],
                        qTp[p][0:64, :],
                        tile_position=(0, 0),
                    )
                    nc.tensor.matmul(
                        psB[:],
                        kTp[p][64:128, 128 * t : 128 * (t + 1)],
                        qTp[p][64:128, :],
                        tile_position=(64, 0),
                    )
                    pA = psbp.tile([128, QH], F32R, tag="psb")
                    pB = psbp.tile([128, QH], F32R, tag="psb")
                    nc.scalar.activation(
                        pA[:], psA[:], AF.Exp, bias=lnG[:, t : t + 1], scale=1.0
                    )
                    nc.scalar.activation(
                        pB[:], psB[:], AF.Exp, bias=lnG[:, t : t + 1], scale=1.0
                    )
                    vv = v_sb[t][:].rearrange("p (h c) -> p h c", c=DH + 1)
                    nc.tensor.matmul(
                        psC_A[:],
                        vv[:, 2 * p, :],
                        pA[:],
                        start=(t == 0),
                        stop=(t == PT - 1),
                    )
                    nc.tensor.matmul(
                        psC_B[:],
                        vv[:, 2 * p + 1, :],
                        pB[:],
                        start=(t == 0),
                        stop=(t == PT - 1),
                    )
                # normalize: ctx rows 0:64 / den row 64
                recA = normp.tile([1, QH], F32R, tag="rec")
                recB = normp.tile([1, QH], F32R, tag="rec")
                with nc.allow_low_precision(reason="f32r recip feeds f32r matmul"):
                    nc.vector.reciprocal(recA[:], psC_A[64:65, :])
                    nc.vector.reciprocal(recB[:], psC_B[64:65, :])
                psD = dpsp.tile([128, QH], F32, tag="dps")
                nc.tensor.matmul(psD[:], selA[0:1, :], recA[0:1, :], start=True, stop=False)
                nc.tensor.matmul(psD[:], selB[0:1, :], recB[0:1, :], start=False, stop=True)
                psD_sb = normp.tile([128, QH], F32, tag="dsb")
                nc.vector.tensor_copy(psD_sb[:], psD[:])
                nc.vector.tensor_mul(
                    ctx_sb[p][0:64, :], psC_A[0:64, :], psD_sb[0:64, :]
                )
                nc.vector.tensor_mul(
                    ctx_sb[p][64:128, :], psC_B[0:64, :], psD_sb[64:128, :]
                )

            # ---- output projection: out = ctx @ Wo + bo ----
            for n in range(2):
                wo_sb = [load_w(Wo, t, n) for t in range(KT)]
                for m in range(QH // 128):
                    ps = accp.tile([128, 512], F32)
                    for t in range(KT):
                        nc.tensor.matmul(
                            ps[:],
                            ctx_sb[t][:, 128 * m : 128 * (m + 1)],
                            wo_sb[t][:],
                            start=(t == 0),
                            stop=False,
                        )
                    nc.tensor.matmul(
                        ps[:],
                        ones[0:1, 0:128],
                        bo_sb[0:1, 512 * n : 512 * (n + 1)],
                        start=False,
                        stop=True,
                    )
                    os_t = osbp.tile([128, 512], F32, tag="osb")
                    nc.vector.tensor_copy(os_t[:], ps[:])
                    nc.sync.dma_start(
                        out=out[128 * m : 128 * (m + 1), 512 * n : 512 * (n + 1)],
                        in_=os_t[:],
                    )

    nc.compile()
    return nc


def kernel(
    query, key, value, mask, gauss_weight, Wq, bq, Wk, bk, Wv, bv, Wo, bo
) -> np.ndarray:
    global LAST_RESULT
    if "nc" not in _CACHED:
        _CACHED["nc"] = _build()
    nc = _CACHED["nc"]

    query = np.asarray(query, dtype=np.float32)
    key = np.asarray(key, dtype=np.float32)
    value = np.asarray(value, dtype=np.float32)
    mask = np.asarray(mask, dtype=np.int32)
    gauss_weight = np.asarray(gauss_weight, dtype=np.float32)
    shared = {
        "Wq": np.ascontiguousarray(Wq, dtype=np.float32),
        "Wk": np.ascontiguousarray(Wk, dtype=np.float32),
        "Wv": np.ascontiguousarray(Wv, dtype=np.float32),
        "Wo": np.ascontiguousarray(Wo, dtype=np.float32),
        "bq": np.ascontiguousarray(bq, dtype=np.float32),
        "bk": np.ascontiguousarray(bk, dtype=np.float32),
        "bv": np.ascontiguousarray(bv, dtype=np.float32),
        "bo": np.ascontiguousarray(bo, dtype=np.float32),
    }

    in_maps = []
    for c in range(N_CORES):
        b, r = c // 2, c % 2
        qTb = np.ascontiguousarray(query[b].T[:, QH * r : QH * (r + 1)])
        in_maps.append(
            {
                "qT": qTb,
                "kT": np.ascontiguousarray(key[b].T),
                "vT": np.ascontiguousarray(value[b].T),
                "gauss": np.ascontiguousarray(gauss_weight[b]),
                "mask": np.ascontiguousarray(mask[b]),
                **shared,
            }
        )

    res = bass_utils.run_bass_kernel_spmd(nc, in_maps, core_ids=list(range(N_CORES)))
    LAST_RESULT = res

    output = np.empty((BS, SEQ, DIM), dtype=np.float32)
    for c in range(N_CORES):
        b, r = c // 2, c % 2
        output[b, QH * r : QH * (r + 1), :] = res.results[c]["out"]
    return output
